# revision 5
# baseline (speedup 1.0000x reference)
"""v14 + mod-128 one-hot: HALF the DVE compare volume.

scratch holds codes&127; one is_equal builds M = onehot(code mod 128).
Per group TWO matmuls share M: P1 = M.T@centLO, P2 = M.T@DIFF
(DIFF = centHI - centLO, host-precomputed bf16), then
    out = P1 + hi * P2,   hi[r, g] = (code >= 128)
computed as two DVE passes (PSUM may appear in at most one operand).
hi is produced in the front phase from the row-layout codes (one tiny
tensor_scalar per chunk).  DVE per block drops from 2x2.35us compares to
1 compare + ~1us of fix-up, shrinking both the DVE pipeline stage and
the window in which GpSimd chains run 4x slow (SBUF contention)."""
import os
import numpy as np
import ml_dtypes
import concourse.bass as bass
import concourse.tile as tile
from concourse import bacc, mybir

B, S = 1024, 200
N_CORES = 8
R = B // N_CORES
D, VALS, SUB = 8, 256, 16
N_ITEMS2 = 1000002
NG = S * D
BLK = 32
CH_T = 16
N_FULL = S // CH_T
LAST_T = S - N_FULL * CH_T
KBUFS = int(os.environ.get("KBUFS", "4"))

_cached = {}


def _build():
    nc = bacc.Bacc("TRN2", target_bir_lowering=False, debug=False,
                   num_devices=N_CORES)
    ids_dram = nc.dram_tensor("input_ids", [R, 2 * S], mybir.dt.int32,
                              kind="ExternalInput").ap()
    codes_dram = nc.dram_tensor("item_codes", [N_ITEMS2, D], mybir.dt.int32,
                                kind="ExternalInput").ap()
    cent_dram = nc.dram_tensor("centb", [16 * 128, SUB], mybir.dt.bfloat16,
                               kind="ExternalInput").ap()
    iota_dram = nc.dram_tensor("iota2", [R, 2], mybir.dt.float32,
                               kind="ExternalInput").ap()
    ident_dram = nc.dram_tensor("ident", [R, 128], mybir.dt.bfloat16,
                                kind="ExternalInput").ap()
    scratch = nc.dram_tensor("scratch", [NG, 128], mybir.dt.uint8).ap()
    out_dram = nc.dram_tensor("out", [R, S * D * SUB], mybir.dt.float32,
                              kind="ExternalOutput").ap()

    chunks = [CH_T] * N_FULL + ([LAST_T] if LAST_T else [])

    with tile.TileContext(nc) as tc:
        with (
            tc.tile_pool(name="const", bufs=1) as cpool,
            tc.tile_pool(name="codes", bufs=KBUFS) as kpool,
            tc.tile_pool(name="oh", bufs=2) as ohpool,
            tc.tile_pool(name="outp", bufs=3) as opool,
            tc.tile_pool(name="psum", bufs=2,
                         space=bass.MemorySpace.PSUM) as pspool,
        ):
            cent = cpool.tile([128, 16 * SUB], mybir.dt.bfloat16)
            nc.sync.dma_start(
                out=cent[:].rearrange("p (s k) -> p s k", k=SUB),
                in_=cent_dram[:].rearrange("(s p) k -> p s k", p=128))
            cent_v = cent[:].rearrange("p (s k) -> p s k", k=SUB)
            iota2 = cpool.tile([R, 2], mybir.dt.float32)
            nc.sync.dma_start(out=iota2[:], in_=iota_dram[:])
            ident = cpool.tile([R, 128], mybir.dt.bfloat16)
            nc.sync.dma_start(out=ident[:], in_=ident_dram[:])

            ids_all = cpool.tile([R, 2 * S], mybir.dt.int32)
            nc.sync.dma_start(out=ids_all[:], in_=ids_dram[:])
            ids32 = cpool.tile([R, S], mybir.dt.int32)
            nc.vector.tensor_copy(
                out=ids32[:],
                in_=ids_all[:].rearrange("p (s two) -> p s two", two=2)[:, :, 0])

            hitiles = {}

            def emit_front(c, T):
                W = T * D
                codes = kpool.tile([R, CH_T * D], mybir.dt.int32, tag="codes")
                for t in range(T):
                    nc.gpsimd.indirect_dma_start(
                        out=codes[:, t * D:(t + 1) * D],
                        out_offset=None,
                        in_=codes_dram[:],
                        in_offset=bass.IndirectOffsetOnAxis(
                            ap=ids32[:, c * CH_T + t:c * CH_T + t + 1],
                            axis=0),
                    )
                # codes mod 128 as bf16 (for the transpose/one-hot) and the
                # high-bit mask as fp32 (for the P1 + hi*P2 fix-up)
                cm32 = kpool.tile([R, CH_T * D], mybir.dt.int32, tag="cm32")
                nc.vector.tensor_scalar(
                    out=cm32[:, :W], in0=codes[:, :W], scalar1=127,
                    scalar2=None, op0=mybir.AluOpType.bitwise_and)
                c16 = kpool.tile([R, CH_T * D], mybir.dt.bfloat16, tag="c16")
                nc.vector.tensor_copy(out=c16[:, :W], in_=cm32[:, :W])
                hi = kpool.tile([R, CH_T * D], mybir.dt.float32, tag="hi")
                nc.vector.tensor_scalar(
                    out=hi[:, :W], in0=codes[:, :W], scalar1=127,
                    scalar2=None, op0=mybir.AluOpType.is_gt)
                hitiles[c] = hi
                pt = pspool.tile([128, 128], mybir.dt.bfloat16, tag="pt")
                nc.tensor.transpose(pt[:W, :], c16[:, :W], ident[:])
                ct8 = kpool.tile([128, 128], mybir.dt.uint8, tag="ct8")
                nc.vector.tensor_copy(out=ct8[:W, :], in_=pt[:W, :])
                nc.sync.dma_start(out=scratch[c * 128:c * 128 + W, :],
                                  in_=ct8[:W, :])

            def emit_blocks(c, T):
                W = T * D
                hi = hitiles[c]
                for nb in range(W // BLK):
                    n = c * 4 + nb
                    cb = ohpool.tile([R, BLK * 128], mybir.dt.uint8, tag="cb")
                    nc.sync.dma_start(
                        out=cb[:],
                        in_=scratch[n * BLK:(n + 1) * BLK, :]
                            .rearrange("g r -> (g r)").unsqueeze(0)
                            .to_broadcast([R, BLK * 128]),
                    )
                    oh = ohpool.tile([R, BLK * 128], mybir.dt.bfloat16,
                                     tag="oh")
                    nc.vector.tensor_scalar(
                        out=oh[:], in0=cb[:], scalar1=iota2[:, 0:1],
                        scalar2=None, op0=mybir.AluOpType.is_equal)
                    ps1 = pspool.tile([128, BLK * SUB], mybir.dt.float32,
                                      tag="ps1")
                    ps2 = pspool.tile([128, BLK * SUB], mybir.dt.float32,
                                      tag="ps2")
                    for g in range(BLK):
                        d = (n * BLK + g) % D
                        nc.tensor.matmul(
                            ps1[:, g * SUB:(g + 1) * SUB],
                            oh[:, g * 128:(g + 1) * 128],
                            cent_v[:, 2 * d, :],
                            start=True, stop=True)
                        nc.tensor.matmul(
                            ps2[:, g * SUB:(g + 1) * SUB],
                            oh[:, g * 128:(g + 1) * 128],
                            cent_v[:, 2 * d + 1, :],
                            start=True, stop=True)
                    # out = P1 + hi * P2  (PSUM in at most one operand/pass)
                    hib = hi[:, nb * BLK:(nb + 1) * BLK] \
                        .unsqueeze(-1).to_broadcast([R, BLK, SUB])
                    tmp = opool.tile([R, BLK * SUB], mybir.dt.float32,
                                     tag="tmp")
                    nc.vector.tensor_tensor(
                        out=tmp[:].rearrange("p (g k) -> p g k", k=SUB),
                        in0=ps2[:].rearrange("p (g k) -> p g k", k=SUB),
                        in1=hib, op=mybir.AluOpType.mult)
                    ot = opool.tile([R, BLK * SUB], mybir.dt.float32,
                                    tag="ot")
                    nc.vector.tensor_tensor(
                        out=ot[:], in0=tmp[:], in1=ps1[:],
                        op=mybir.AluOpType.add)
                    nc.sync.dma_start(
                        out=out_dram[:, n * BLK * SUB:(n + 1) * BLK * SUB],
                        in_=ot[:])

            emit_front(0, chunks[0])
            for c in range(1, len(chunks)):
                emit_front(c, chunks[c])
                emit_blocks(c - 1, chunks[c - 1])
            emit_blocks(len(chunks) - 1, chunks[-1])
    nc.compile()
    return nc


def _get_nc():
    if "nc" not in _cached:
        _cached["nc"] = _build()
    return _cached["nc"]


def _host_consts(centroids):
    cent = np.asarray(centroids, dtype=np.float32)
    centb = cent.astype(ml_dtypes.bfloat16)
    # slot 2d   -> centLO = bf16(cent[d, 0:128])
    # slot 2d+1 -> DIFF   = bf16(cent[d, 128:256] - cent[d, 0:128]) as fp32 sub
    cb = np.zeros((16 * 128, SUB), dtype=ml_dtypes.bfloat16)
    for d in range(D):
        lo = centb[d, 0:128].astype(np.float32)
        hi = centb[d, 128:256].astype(np.float32)
        cb[(d * 2) * 128:(d * 2 + 1) * 128] = lo.astype(ml_dtypes.bfloat16)
        cb[(d * 2 + 1) * 128:(d * 2 + 2) * 128] = \
            (hi - lo).astype(ml_dtypes.bfloat16)
    iota2 = np.stack([np.arange(128, dtype=np.float32),
                      np.arange(128, 256, dtype=np.float32)], axis=1)
    ident = np.eye(128, dtype=ml_dtypes.bfloat16)
    return np.ascontiguousarray(cb), iota2, np.ascontiguousarray(ident)


def kernel(input_ids, item_codes, centroids, _debug_run_kwargs=None):
    from concourse.bass_utils import run_bass_kernel_spmd

    nc = _get_nc()
    input_ids = np.ascontiguousarray(np.asarray(input_ids, dtype=np.int64))
    item_codes = np.ascontiguousarray(np.asarray(item_codes, dtype=np.int32))
    cb, iota2, ident = _host_consts(centroids)

    in_maps = [
        {
            "input_ids": np.ascontiguousarray(
                input_ids[c * R:(c + 1) * R]).view(np.int32),
            "item_codes": item_codes,
            "centb": cb,
            "iota2": iota2,
            "ident": ident,
        }
        for c in range(N_CORES)
    ]
    res = run_bass_kernel_spmd(nc, in_maps, list(range(N_CORES)),
                               **(_debug_run_kwargs or {}))
    if _debug_run_kwargs:
        _cached["last_results"] = res
    out = np.concatenate(
        [res.results[c]["out"].reshape(R, S, D * SUB)
         for c in range(N_CORES)], axis=0)
    return out


# revision 6
# speedup vs baseline: 1.1905x; 1.1905x over previous
"""v14 + mod-128 one-hot: HALF the DVE compare volume.

scratch holds codes&127; one is_equal builds M = onehot(code mod 128).
Per group TWO matmuls share M: P1 = M.T@centLO, P2 = M.T@DIFF
(DIFF = centHI - centLO, host-precomputed bf16), then
    out = P1 + hi * P2,   hi[r, g] = (code >= 128)
computed as two DVE passes (PSUM may appear in at most one operand).
hi is produced in the front phase from the row-layout codes (one tiny
tensor_scalar per chunk).  DVE per block drops from 2x2.35us compares to
1 compare + ~1us of fix-up, shrinking both the DVE pipeline stage and
the window in which GpSimd chains run 4x slow (SBUF contention)."""
import os
import numpy as np
import ml_dtypes
import concourse.bass as bass
import concourse.tile as tile
from concourse import bacc, mybir

B, S = 1024, 200
N_CORES = 8
R = B // N_CORES
D, VALS, SUB = 8, 256, 16
N_ITEMS2 = 1000002
NG = S * D
BLK = 32
CH_T = 16
N_FULL = S // CH_T
LAST_T = S - N_FULL * CH_T
KBUFS = int(os.environ.get("KBUFS", "4"))

_cached = {}


def _build():
    nc = bacc.Bacc("TRN2", target_bir_lowering=False, debug=False,
                   num_devices=N_CORES)
    ids_dram = nc.dram_tensor("input_ids", [R, 2 * S], mybir.dt.int32,
                              kind="ExternalInput").ap()
    codes_dram = nc.dram_tensor("item_codes", [N_ITEMS2, D], mybir.dt.int32,
                                kind="ExternalInput").ap()
    cent_dram = nc.dram_tensor("centb", [16 * 128, SUB], mybir.dt.bfloat16,
                               kind="ExternalInput").ap()
    iota_dram = nc.dram_tensor("iota2", [R, 2], mybir.dt.float32,
                               kind="ExternalInput").ap()
    ident_dram = nc.dram_tensor("ident", [R, 128], mybir.dt.bfloat16,
                                kind="ExternalInput").ap()
    scratch = nc.dram_tensor("scratch", [NG, 128], mybir.dt.uint8).ap()
    out_dram = nc.dram_tensor("out", [R, S * D * SUB], mybir.dt.float32,
                              kind="ExternalOutput").ap()

    chunks = [CH_T] * N_FULL + ([LAST_T] if LAST_T else [])

    with tile.TileContext(nc) as tc:
        with (
            tc.tile_pool(name="const", bufs=1) as cpool,
            tc.tile_pool(name="codes", bufs=KBUFS) as kpool,
            tc.tile_pool(name="oh", bufs=2) as ohpool,
            tc.tile_pool(name="outp", bufs=3) as opool,
            tc.tile_pool(name="psum", bufs=2,
                         space=bass.MemorySpace.PSUM) as pspool,
        ):
            cent = cpool.tile([128, 16 * SUB], mybir.dt.bfloat16)
            nc.sync.dma_start(
                out=cent[:].rearrange("p (s k) -> p s k", k=SUB),
                in_=cent_dram[:].rearrange("(s p) k -> p s k", p=128))
            cent_v = cent[:].rearrange("p (s k) -> p s k", k=SUB)
            iota2 = cpool.tile([R, 2], mybir.dt.float32)
            nc.sync.dma_start(out=iota2[:], in_=iota_dram[:])
            ident = cpool.tile([R, 128], mybir.dt.bfloat16)
            nc.sync.dma_start(out=ident[:], in_=ident_dram[:])

            ids_all = cpool.tile([R, 2 * S], mybir.dt.int32)
            nc.sync.dma_start(out=ids_all[:], in_=ids_dram[:])
            ids32 = cpool.tile([R, S], mybir.dt.int32)
            nc.vector.tensor_copy(
                out=ids32[:],
                in_=ids_all[:].rearrange("p (s two) -> p s two", two=2)[:, :, 0])

            hitiles = {}

            def emit_front(c, T):
                W = T * D
                codes = kpool.tile([R, CH_T * D], mybir.dt.int32, tag="codes")
                for t in range(T):
                    nc.gpsimd.indirect_dma_start(
                        out=codes[:, t * D:(t + 1) * D],
                        out_offset=None,
                        in_=codes_dram[:],
                        in_offset=bass.IndirectOffsetOnAxis(
                            ap=ids32[:, c * CH_T + t:c * CH_T + t + 1],
                            axis=0),
                    )
                # codes mod 128 as bf16 (for the transpose/one-hot) and the
                # high-bit mask as fp32 (for the P1 + hi*P2 fix-up)
                cm32 = kpool.tile([R, CH_T * D], mybir.dt.int32, tag="cm32")
                nc.vector.tensor_scalar(
                    out=cm32[:, :W], in0=codes[:, :W], scalar1=127,
                    scalar2=None, op0=mybir.AluOpType.bitwise_and)
                c16 = kpool.tile([R, CH_T * D], mybir.dt.bfloat16, tag="c16")
                nc.vector.tensor_copy(out=c16[:, :W], in_=cm32[:, :W])
                hi = kpool.tile([R, CH_T * D], mybir.dt.float32, tag="hi")
                nc.vector.tensor_scalar(
                    out=hi[:, :W], in0=codes[:, :W], scalar1=127,
                    scalar2=None, op0=mybir.AluOpType.is_gt)
                hitiles[c] = hi
                pt = pspool.tile([128, 128], mybir.dt.bfloat16, tag="pt")
                nc.tensor.transpose(pt[:W, :], c16[:, :W], ident[:])
                ct8 = kpool.tile([128, 128], mybir.dt.uint8, tag="ct8")
                nc.vector.tensor_copy(out=ct8[:W, :], in_=pt[:W, :])
                nc.sync.dma_start(out=scratch[c * 128:c * 128 + W, :],
                                  in_=ct8[:W, :])

            def emit_blocks(c, T):
                W = T * D
                hi = hitiles[c]
                for nb in range(W // BLK):
                    n = c * 4 + nb
                    cb = ohpool.tile([R, BLK * 128], mybir.dt.uint8, tag="cb")
                    nc.sync.dma_start(
                        out=cb[:],
                        in_=scratch[n * BLK:(n + 1) * BLK, :]
                            .rearrange("g r -> (g r)").unsqueeze(0)
                            .to_broadcast([R, BLK * 128]),
                    )
                    oh = ohpool.tile([R, BLK * 128], mybir.dt.bfloat16,
                                     tag="oh")
                    nc.vector.tensor_scalar(
                        out=oh[:], in0=cb[:], scalar1=iota2[:, 0:1],
                        scalar2=None, op0=mybir.AluOpType.is_equal)
                    # one matmul per group: rhs = [centLO_d | DIFF_d]
                    # (adjacent slots) -> PSUM [.., g, 2, 16] = [P1 | P2]
                    ps = pspool.tile([128, BLK * 2 * SUB], mybir.dt.float32,
                                     tag="ps")
                    for g in range(BLK):
                        d = (n * BLK + g) % D
                        nc.tensor.matmul(
                            ps[:, g * 2 * SUB:(g + 1) * 2 * SUB],
                            oh[:, g * 128:(g + 1) * 128],
                            cent_v[:, 2 * d:2 * d + 2, :],
                            start=True, stop=True)
                    # out = P1 + hi * P2  (PSUM in at most one operand/pass)
                    ps_v = ps[:].rearrange("p (g two k) -> p g two k",
                                           two=2, k=SUB)
                    hib = hi[:, nb * BLK:(nb + 1) * BLK] \
                        .unsqueeze(-1).to_broadcast([R, BLK, SUB])
                    tmp = opool.tile([R, BLK * SUB], mybir.dt.float32,
                                     tag="tmp")
                    nc.vector.tensor_tensor(
                        out=tmp[:].rearrange("p (g k) -> p g k", k=SUB),
                        in0=ps_v[:, :, 1, :],
                        in1=hib, op=mybir.AluOpType.mult)
                    ot = opool.tile([R, BLK * SUB], mybir.dt.float32,
                                    tag="ot")
                    nc.vector.tensor_tensor(
                        out=ot[:].rearrange("p (g k) -> p g k", k=SUB),
                        in0=tmp[:].rearrange("p (g k) -> p g k", k=SUB),
                        in1=ps_v[:, :, 0, :],
                        op=mybir.AluOpType.add)
                    nc.sync.dma_start(
                        out=out_dram[:, n * BLK * SUB:(n + 1) * BLK * SUB],
                        in_=ot[:])

            emit_front(0, chunks[0])
            for c in range(1, len(chunks)):
                emit_front(c, chunks[c])
                emit_blocks(c - 1, chunks[c - 1])
            emit_blocks(len(chunks) - 1, chunks[-1])
    nc.compile()
    return nc


def _get_nc():
    if "nc" not in _cached:
        _cached["nc"] = _build()
    return _cached["nc"]


def _host_consts(centroids):
    cent = np.asarray(centroids, dtype=np.float32)
    centb = cent.astype(ml_dtypes.bfloat16)
    # slot 2d   -> centLO = bf16(cent[d, 0:128])
    # slot 2d+1 -> DIFF   = bf16(cent[d, 128:256] - cent[d, 0:128]) as fp32 sub
    cb = np.zeros((16 * 128, SUB), dtype=ml_dtypes.bfloat16)
    for d in range(D):
        lo = centb[d, 0:128].astype(np.float32)
        hi = centb[d, 128:256].astype(np.float32)
        cb[(d * 2) * 128:(d * 2 + 1) * 128] = lo.astype(ml_dtypes.bfloat16)
        cb[(d * 2 + 1) * 128:(d * 2 + 2) * 128] = \
            (hi - lo).astype(ml_dtypes.bfloat16)
    iota2 = np.stack([np.arange(128, dtype=np.float32),
                      np.arange(128, 256, dtype=np.float32)], axis=1)
    ident = np.eye(128, dtype=ml_dtypes.bfloat16)
    return np.ascontiguousarray(cb), iota2, np.ascontiguousarray(ident)


def kernel(input_ids, item_codes, centroids, _debug_run_kwargs=None):
    from concourse.bass_utils import run_bass_kernel_spmd

    nc = _get_nc()
    input_ids = np.ascontiguousarray(np.asarray(input_ids, dtype=np.int64))
    item_codes = np.ascontiguousarray(np.asarray(item_codes, dtype=np.int32))
    cb, iota2, ident = _host_consts(centroids)

    in_maps = [
        {
            "input_ids": np.ascontiguousarray(
                input_ids[c * R:(c + 1) * R]).view(np.int32),
            "item_codes": item_codes,
            "centb": cb,
            "iota2": iota2,
            "ident": ident,
        }
        for c in range(N_CORES)
    ]
    res = run_bass_kernel_spmd(nc, in_maps, list(range(N_CORES)),
                               **(_debug_run_kwargs or {}))
    if _debug_run_kwargs:
        _cached["last_results"] = res
    out = np.concatenate(
        [res.results[c]["out"].reshape(R, S, D * SUB)
         for c in range(N_CORES)], axis=0)
    return out
